# revision 49
# baseline (speedup 1.0000x reference)
"""Trainium2 Bass kernel for single-step causal GQA attention with KV cache.

Problem (hardcoded shapes):
  x[4,16,4096] @ Wq/Wk/Wv -> RoPE -> append to KV cache (start_pos=2048) ->
  GQA attention over T=2064 keys -> @ Wo -> out[4,16,4096], all fp32.

Sharding (8 cores, tensor-parallel over heads):
  core c gets q-heads 4c..4c+3 (Wq cols c*512:(c+1)*512), kv-head c
  (Wk/Wv cols c*128:(c+1)*128, cache [:, :, c, :]), and Wo rows
  c*512:(c+1)*512 (row-parallel). The 8 partial [64,4096] outputs are
  summed on the host.

The kernel is DMA-bound (~15.5 MB/core of weights+cache), so everything
streamed from HBM is bf16: weights, caches, x, and rope tables are
converted on the host; matmuls run with bf16 operands (1 PE cycle/row vs
4 for fp32) accumulating into fp32 PSUM. All DMAs keep >=512B contiguous
runs (below that the bus pays 2x), which is why V is host-padded to
HD+4 columns with the softmax-denominator ones column baked in.

Per-core layout choices:
  - x is pre-transposed on host to xT[4096,64] so the QKV contraction
    (over DIM) streams W tiles as the moving operand with xT chunks as
    the 64-col stationary.
  - K cache is pre-transposed on host to kT[4,128,2048] so score matmuls
    use kT chunks [128(hd),128(t)] directly as stationary; scores come
    out transposed: scoresT[t, (h,s)] with keys on partitions.
  - softmax: exp without max-subtraction (|scores|<~10 for this data
    regime, exp is exact enough), and the denominator comes from a ones
    column in the V tiles, accumulated next to the unnormalized
    attention output, so no cross-partition reduction is ever needed.
  - RoPE is two multiplies and an add using host-precomputed cos/sin
    broadcast tiles, with the pair-swap expressed as a negative-stride AP.
"""

import math
import os

import numpy as np
from ml_dtypes import bfloat16

# SWDGE-scatter final flush (fast tail) vs plain HWDGE flushes (safe).
# The scatter path passes CoreSim/TimelineSim but hangs the real device
# (its DMA-completion sem wiring under walrus differs), so default off.
USE_SCATTER = os.environ.get("KERNEL_SCATTER", "0") == "1"
# zero-stride broadcast APs for the per-head rope tables vs host-materialized
ROPE_BCAST = os.environ.get("KERNEL_ROPE_BCAST", "1") == "1"

import concourse.bass as bass
import concourse.mybir as mybir
import concourse.tile as tile
from concourse import bacc
from concourse.bass_utils import run_bass_kernel_spmd
from concourse.masks import make_identity

F32 = mybir.dt.float32
BF16 = mybir.dt.bfloat16

B, S, DIM = 4, 16, 4096
NH, NKV, HD = 32, 8, 128
START = 2048
BS = B * S              # 64 tokens
NCORES = 8
QH = NH // NCORES       # 4 q heads per core
QW = QH * HD            # 512 = per-core Wq width
KC = DIM // 128         # 32 contraction chunks
NT = START // 128       # 16 full cache chunks per batch
TW = QH * S             # 64 = scoresT free width (h-major, then s)
EXPW = NT * TW          # 1024 = expT tile width (16 cache chunks)
VW = HD + 1             # padded V row: 128 v cols + ones col
SCALE = 1.0 / math.sqrt(HD)


def swap_pairs(ap):
    # [p, w] view with even/odd element pairs swapped: (x1,x0,x3,x2,...)
    p, w = ap.shape
    return bass.AP(ap.tensor, ap.offset + 1, [ap.ap[0], [2, w // 2], [-1, 2]])


def bcast_mid(ap2d, reps):
    # [p, w] view broadcast to [p, reps, w] via a zero-stride middle dim
    return bass.AP(ap2d.tensor, ap2d.offset, [ap2d.ap[0], [0, reps], ap2d.ap[1]])


def build_bass() -> bass.Bass:
    nc = bacc.Bacc()

    # xT is host-laid-out as [128(p), KC*64(c,s)] = exact SBUF order,
    # so the load is fully contiguous (8KB rows)
    xT = nc.dram_tensor("xT", [128, KC * BS], BF16, kind="ExternalInput")
    wq = nc.dram_tensor("wq", [128, KC * QW], BF16, kind="ExternalInput")
    wk = nc.dram_tensor("wk", [128, KC * HD], BF16, kind="ExternalInput")
    wv = nc.dram_tensor("wv", [128, KC * HD], BF16, kind="ExternalInput")
    wo = nc.dram_tensor("wo", [128, 8, 4 * 512], BF16, kind="ExternalInput")
    kT = nc.dram_tensor("kT", [B, HD, START], BF16, kind="ExternalInput")
    # V pre-padded on host to [.., NT*(HD+4)] with the ones column at HD
    vc = nc.dram_tensor("vc", [B, 128, NT * VW], BF16, kind="ExternalInput")
    # rope = [cc-base | ss-base | batch mask] merged into one load; with
    # ROPE_BCAST the per-head cc/ss broadcast happens via zero-stride APs
    # on chip, otherwise the host materializes the full-width tables
    ROPE_W = (2 * HD if ROPE_BCAST else 2 * QW + 2 * HD) + QH * BS
    rope = nc.dram_tensor("rope", [BS, ROPE_W], BF16, kind="ExternalInput")
    # scatter indices for the final out columns: row j at [j%16, j//16],
    # -1 elsewhere (scatter rows 64.. are skipped)
    idx16 = nc.dram_tensor("idx16", [128, 8], mybir.dt.int16,
                           kind="ExternalInput")
    out = nc.dram_tensor("out", [BS, DIM], BF16, kind="ExternalOutput")

    with tile.TileContext(nc) as tc:
        with (
            tc.tile_pool(name="const", bufs=1) as const,
            tc.tile_pool(name="wqp", bufs=8) as wqp,
            tc.tile_pool(name="wkvp", bufs=1) as wkvp,
            tc.tile_pool(name="wop", bufs=12) as wop,
            tc.tile_pool(name="kvp", bufs=4) as kvp,
            tc.tile_pool(name="acts", bufs=1) as acts,
            tc.tile_pool(name="expp", bufs=2) as expp,
            tc.tile_pool(name="small", bufs=4) as small,
        ):
            ident = const.tile([128, 64], F32, tag="ident")
            make_identity(nc, ident[:64, :])
            # touch Exp once so the ACT LUT loads during phase 1, not on
            # the batch-0 softmax critical path
            warm = const.tile([1, 4], F32, tag="warm")
            nc.scalar.activation(
                warm[:], ident[:1, :4], mybir.ActivationFunctionType.Exp
            )

            # DMA issue order matters: per-lane FIFO means earlier-needed
            # tiles must be enqueued first (xT, rope consts, wq stream);
            # wk/wv follow; kT/v and the big wo tiles go inside the batch
            # loop so they fill DMA idle time without delaying the start.
            xT_sb = const.tile([128, KC, BS], BF16, tag="xT")
            x4 = xT.ap()

            idx_sb = const.tile([128, 8], mybir.dt.int16, tag="idx16")
            rope_sb = const.tile([BS, ROPE_W], BF16, tag="rope")
            if ROPE_BCAST:
                cck_sb = rope_sb[:, 0:HD]
                ssk_sb = rope_sb[:, HD:2 * HD]
                mask_sb = rope_sb[:, 2 * HD:]      # [64, QH*64] batch mask
                ccq_sb = bcast_mid(cck_sb, QH)     # [64, QH, HD] broadcast
                ssq_sb = bcast_mid(ssk_sb, QH)
            else:
                ccq_sb = rope_sb[:, 0:QW]
                ssq_sb = rope_sb[:, QW:2 * QW]
                cck_sb = rope_sb[:, 2 * QW:2 * QW + HD]
                ssk_sb = rope_sb[:, 2 * QW + HD:2 * QW + 2 * HD]
                mask_sb = rope_sb[:, 2 * QW + 2 * HD:]

            wo_sb = []
            attnT = acts.tile([128, QH, BS], BF16, tag="attnT")

            with tc.tile_pool(name="ps_t", bufs=2, space="PSUM") as ps_t:
                # ---- QKV projections: psum[tok, width] += xT_c.T @ W_c ----
                with tc.tile_pool(name="ps_qkv", bufs=1, space="PSUM") as ps_qkv:
                    xq_ps = ps_qkv.tile([BS, QW], F32, tag="xq")
                    xk_ps = ps_qkv.tile([BS, HD], F32, tag="xk")
                    xv_ps = ps_qkv.tile([BS, HD], F32, tag="xv")

                    # interleave xT piece / wq group DMAs so the first
                    # matmul's inputs arrive early; 4 chunks per group keeps
                    # every contiguous run >= 512B in bf16
                    # wq group issued before its xT piece: the first (long)
                    # wq transfer covers the second DMA's descriptor-gen
                    # latency, so the bus never idles at the start
                    wq3 = wq.ap()
                    for g in range(8):  # 4 chunks per DMA group
                        wq_sb = wqp.tile([128, 4, QW], BF16, tag="wq")
                        nc.sync.dma_start(wq_sb[:], wq3[:, 2048 * g : 2048 * (g + 1)])
                        nc.sync.dma_start(
                            xT_sb[:, 4 * g : 4 * (g + 1), :],
                            x4[:, 256 * g : 256 * (g + 1)],
                        )
                        if g == 4:  # consts mid-stream: by then the bus is
                            # ahead of HWDGE descriptor-gen, so these two
                            # cheap DMAs don't delay the wq pipeline
                            nc.sync.dma_start(rope_sb[:], rope.ap())
                            nc.sync.dma_start(idx_sb[:], idx16.ap())
                        for j in range(4):
                            c = 4 * g + j
                            nc.tensor.matmul(
                                xq_ps[:],
                                lhsT=xT_sb[:, c, :],
                                rhs=wq_sb[:, j, :],
                                start=(c == 0),
                                stop=(c == KC - 1),
                            )

                    wk_sb = wkvp.tile([128, KC, HD], BF16, tag="wk")
                    nc.sync.dma_start(wk_sb[:], wk.ap())
                    wv_sb = wkvp.tile([128, KC, HD], BF16, tag="wv")
                    nc.sync.dma_start(wv_sb[:], wv.ap())
                    for c in range(KC):
                        nc.tensor.matmul(
                            xk_ps[:],
                            lhsT=xT_sb[:, c, :],
                            rhs=wk_sb[:, c, :],
                            start=(c == 0),
                            stop=(c == KC - 1),
                        )
                    for c in range(KC):
                        nc.tensor.matmul(
                            xv_ps[:],
                            lhsT=xT_sb[:, c, :],
                            rhs=wv_sb[:, c, :],
                            start=(c == 0),
                            stop=(c == KC - 1),
                        )

                    # ---- RoPE (token-major): o = x*cc + swap(x)*ss ----
                    xq_sb = acts.tile([BS, QW], F32, tag="xq_sb")
                    tq = acts.tile([BS, QW], F32, tag="tq")
                    nc.vector.tensor_mul(xq_sb[:], xq_ps[:], ccq_sb)
                    nc.vector.tensor_mul(tq[:], swap_pairs(xq_ps[:]), ssq_sb)
                    nc.vector.tensor_add(xq_sb[:], xq_sb[:], tq[:])

                    xk_sb = acts.tile([BS, HD], F32, tag="xk_sb")
                    tk = acts.tile([BS, HD], F32, tag="tk")
                    nc.vector.tensor_mul(xk_sb[:], xk_ps[:], cck_sb)
                    nc.vector.tensor_mul(tk[:], swap_pairs(xk_ps[:]), ssk_sb)
                    nc.vector.tensor_add(xk_sb[:], xk_sb[:], tk[:])

                    # v_new, token-major [64(b,s), HD | ones col]: used with a
                    # batch-masked exp so the contraction over all 64 token
                    # partitions only picks up the right batch's rows
                    xv1_sb = acts.tile([BS, HD + 1], BF16, tag="xv1")
                    nc.vector.tensor_copy(xv1_sb[:, :HD], xv_ps[:])
                    nc.vector.memset(xv1_sb[:, HD : HD + 1], 1.0)

                    # ---- transposes: qT [hd, (h | b,s)], kT_new [hd, (b,s)]
                    # (fp32 through PSUM; the copy out converts to bf16)
                    qT_sb = acts.tile([128, QH, TW], BF16, tag="qT")
                    for h in range(QH):
                        ps = ps_t.tile([128, BS], F32, tag="tr")
                        nc.tensor.transpose(
                            ps[:], xq_sb[:, 128 * h : 128 * (h + 1)],
                            ident[:BS, :BS],
                        )
                        nc.vector.tensor_copy(qT_sb[:, h, :], ps[:])
                    kTn_sb = acts.tile([128, BS], BF16, tag="kTn")
                    psn = ps_t.tile([128, BS], F32, tag="tr")
                    nc.tensor.transpose(psn[:], xk_sb[:], ident[:BS, :BS])
                    nc.vector.tensor_copy(kTn_sb[:], psn[:])

                    # ---- new-token scores for ALL batches in one matmul:
                    # scn_all[(bk,sk), (h,bq,sq)] then exp, then zero the
                    # bq != bk blocks with a host-precomputed 0/1 mask.
                    # expn is laid out batch-major so each batch's lhsT
                    # slice is one contiguous free dim (walrus requires
                    # single-free-dim stationary APs); exp runs per batch
                    # reading the (h, b, s)-ordered scores via a strided AP.
                    scn_ps = ps_qkv.tile([BS, QH * BS], F32, tag="scn")
                    nc.tensor.matmul(
                        scn_ps[:],
                        lhsT=kTn_sb[:],
                        rhs=qT_sb[:, :, :],
                        start=True,
                        stop=True,
                    )
                    expn = acts.tile([BS, B, QH * S], BF16, tag="expn")
                    scn_ap = scn_ps[:]
                    for b in range(B):
                        scn_b = bass.AP(
                            scn_ap.tensor, scn_ap.offset + S * b,
                            [scn_ap.ap[0], [TW, QH], [1, S]],
                        )
                        nc.scalar.activation(
                            expn[:, b, :], scn_b,
                            mybir.ActivationFunctionType.Exp,
                            scale=SCALE,
                        )
                    nc.vector.tensor_mul(expn[:], expn[:], mask_sb)

                # ---- attention per batch ----
                with (
                    tc.tile_pool(name="ps_sc", bufs=3, space="PSUM") as ps_sc,
                    tc.tile_pool(name="ps_ou", bufs=2, space="PSUM") as ps_ou,
                ):
                    def load_kv(b):
                        kT_sb = kvp.tile([128, START], BF16, tag="kT",
                                         name="kT_sb")
                        nc.sync.dma_start(kT_sb[:], kT.ap()[b])
                        v_sb = kvp.tile([128, NT * VW], BF16, tag="v",
                                        name="v_sb")
                        nc.sync.dma_start(v_sb[:], vc.ap()[b])
                        return kT_sb, v_sb

                    # all four batches' K/V prefetch up front (kvp bufs=4):
                    # the late batches' chains are the kernel tail, so their
                    # data must never wait behind the Wo prefetches
                    kv_tiles = {b: load_kv(b) for b in range(B)}
                    for b in range(B):
                        kT_sb, v_sb = kv_tiles.pop(b)

                        qT_b = qT_sb[:, :, 16 * b : 16 * (b + 1)]  # [128,4,16]

                        expT = expp.tile([128, EXPW], BF16, tag="expT")
                        for u in range(NT // 4):  # one exp per 4 chunks
                            sc = ps_sc.tile([128, 4, TW], F32, tag="sc")
                            for j in range(4):
                                t = 4 * u + j
                                nc.tensor.matmul(
                                    sc[:, j, :],
                                    lhsT=kT_sb[:, 128 * t : 128 * (t + 1)],
                                    rhs=qT_b,
                                    start=True,
                                    stop=True,
                                )
                            nc.scalar.activation(
                                expT[:, 4 * TW * u : 4 * TW * (u + 1)],
                                sc[:],
                                mybir.ActivationFunctionType.Exp,
                                scale=SCALE,
                            )
                        # unnormalized out [tok(h,s), hd | exp-sum col at HD]
                        ou = ps_ou.tile([TW, VW], F32, tag="ou")
                        for t in range(NT):
                            nc.tensor.matmul(
                                ou[:, : HD + 1],
                                lhsT=expT[:, TW * t : TW * (t + 1)],
                                rhs=v_sb[:, VW * t : VW * t + HD + 1],
                                start=(t == 0),
                                stop=False,
                            )
                        nc.tensor.matmul(
                            ou[:, : HD + 1],
                            lhsT=expn[:, b, :],
                            rhs=xv1_sb[:],
                            start=False,
                            stop=True,
                        )

                        rcp = small.tile([TW, 1], F32, tag="rcp")
                        nc.vector.reciprocal(rcp[:], ou[:, HD : HD + 1])
                        attn = small.tile([TW, HD], F32, tag="attn")
                        nc.vector.tensor_scalar_mul(attn[:], ou[:, :HD], rcp[:])

                        aps = ps_t.tile([128, TW], F32, tag="tr")
                        nc.tensor.transpose(aps[:], attn[:], ident[:TW, :TW])
                        # one strided copy: dst (h, s) columns <- src h-major
                        nc.vector.tensor_copy(
                            attnT[:, :, 16 * b : 16 * (b + 1)], aps[:]
                        )

                        # Wo prefetch as column blocks, two per batch; the
                        # final blocks narrow progressively (block 6 halves,
                        # block 7 quarters, host-relaid to stay contiguous)
                        # so the last-arriving bytes have the shortest
                        # possible downstream chain (4 matmuls of N=128)
                        if b < 3:
                            for j in (2 * b, 2 * b + 1):
                                wo_t = wop.tile([128, 4, 512], BF16, tag="wo",
                                                name="wo_t")
                                nc.sync.dma_start(wo_t[:], wo.ap()[:, j, :])
                                wo_sb.append(wo_t)
                        else:
                            for h in (0, 1):
                                wo_t = wop.tile([128, 4, 256], BF16,
                                                tag="wo", name="wo_t")
                                nc.sync.dma_start(
                                    wo_t[:],
                                    wo.ap()[:, 6, 1024 * h : 1024 * (h + 1)],
                                )
                                wo_sb.append(wo_t)
                            for h in range(4):
                                wo_t = wop.tile([128, 4, 128], BF16,
                                                tag="wo", name="wo_t")
                                nc.sync.dma_start(
                                    wo_t[:],
                                    wo.ap()[:, 7, 512 * h : 512 * (h + 1)],
                                )
                                wo_sb.append(wo_t)

            # ---- output projection: out[64, 4096] = attnT.T @ Wo ----
            # n-outer: out n-tile j consumes only column block j, freeing
            # its slot for blocks 6/7 to load as soon as attnT completes
            with (
                tc.tile_pool(name="outp", bufs=2) as outp,
                tc.tile_pool(name="ps_wo", bufs=3, space="PSUM") as ps_wo,
            ):
                # (block, n-columns within out, width)
                pieces = [(i, 512 * i, 512) for i in range(6)] + [
                    (6, 3072, 256), (7, 3328, 256),
                    (8, 3584, 128), (9, 3712, 128),
                    (10, 3840, 128), (11, 3968, 128),
                ]
                o_halves = [
                    outp.tile([BS, 2048], BF16, tag="o", name="o_sb")
                    for _ in range(2)
                ]
                # Columns [3072:4096] go out via a pre-prepared SWDGE scatter
                # (64 row descriptors of 2KB, fired by a cheap Pool trigger
                # after the last copy) instead of an HWDGE dma_start — this
                # skips the ~1.3us descriptor-gen pipeline that would
                # otherwise sit on the critical path after the last Wo bytes.
                if USE_SCATTER:
                    stage = outp.tile([128, 1, 1024], BF16, tag="stage")
                    nc.gpsimd.memset(stage[:], 0.0)
                    scat_sem = nc.alloc_semaphore("scat_done")
                    nc.gpsimd.dma_scatter_add(
                        out.ap()[:, 3072:4096],
                        stage[:],
                        idx_sb[:, :4],
                        num_idxs=BS,
                        num_idxs_reg=BS,
                        elem_size=1024,
                        elem_step=DIM,
                        prepare_only=True,
                        sem=scat_sem,
                    )
                for i, (blk, col, w) in enumerate(pieces):
                    wo_ps = ps_wo.tile([BS, 512], F32, tag="wops", name="wo_ps")
                    for k in range(4):
                        nc.tensor.matmul(
                            wo_ps[:, :w],
                            lhsT=attnT[:, k, :],
                            rhs=wo_sb[blk][:, k, :],
                            start=(k == 0),
                            stop=(k == 3),
                        )
                    eng = nc.vector.tensor_copy if i % 2 == 0 else nc.scalar.copy
                    if USE_SCATTER and col >= 3072:
                        eng(stage[:BS, 0, col - 3072 : col - 3072 + w],
                            wo_ps[:, :w])
                    else:
                        o_sb = o_halves[col // 2048]
                        eng(o_sb[:, col % 2048 : col % 2048 + w], wo_ps[:, :w])
                    if col + w == 2048:
                        nc.sync.dma_start(out.ap()[:, :2048], o_sb[:])
                    elif col + w == 3072:
                        nc.sync.dma_start(
                            out.ap()[:, 2048:3072], o_sb[:, :1024]
                        )
                    elif col + w == 3584 and not USE_SCATTER:
                        nc.sync.dma_start(
                            out.ap()[:, 3072:3584], o_sb[:, 1024:1536]
                        )
                    elif col + w == 3968 and not USE_SCATTER:
                        nc.sync.dma_start(
                            out.ap()[:, 3584:3968], o_sb[:, 1536:1920]
                        )
                    elif col + w == 4096:
                        if USE_SCATTER:
                            nc.gpsimd.trigger_dma(count=None)
                        else:
                            # on ACT, right after the (ACT) piece-11 copy:
                            # same-engine ordering skips the cross-engine
                            # sem and SP's SEQ serialization
                            nc.scalar.dma_start(
                                out.ap()[:, 3968:4096], o_sb[:, 1920:]
                            )
                if USE_SCATTER:
                    nc.gpsimd.wait_ge(scat_sem, 16)

    nc.compile()

    # Tile's teardown drain waits on the SWDGE lane sem (DMASW0) that the
    # prepared scatter's tick advanced — but a gen_mode=1 prep's completion
    # fires its custom sem= instead, so that wait can never be satisfied
    # (framework gap). The explicit gpsimd.wait_ge(scat_sem) above already
    # holds the final barrier until the scatter lands, so the orphaned
    # DMASW wait is redundant: drop it.
    if USE_SCATTER:
        patched = 0
        for blk in nc.m.functions[0].blocks:
            for inst in blk.instructions:
                si = inst.sync_info
                if si is None:
                    continue
                waits = list(si.on_wait)
                kept = [
                    w for w in waits
                    if not (w.ant_name or "").startswith("DMASW")
                ]
                if len(kept) != len(waits):
                    si.on_wait = kept
                    patched += 1
        assert patched == 1, (
            f"expected exactly one DMASW drain wait, {patched=}"
        )
    return nc


def _rope_mask_tensor(freqs_cos, freqs_sin):
    # cc/ss rope tiles (row r=(b*16+s), col 2i+j; o = x*cc + swap(x)*ss
    # with cc=[c,c,...], ss=[-s,+s,...]) then the batch mask
    # m[(bk,sk),(h,bq,sq)] = (bk==bq).
    cos = np.asarray(freqs_cos, np.float32)  # [S, 64]
    sin = np.asarray(freqs_sin, np.float32)
    cc1 = np.repeat(cos, 2, axis=1)  # [S, 128]
    ss1 = np.repeat(sin, 2, axis=1).copy()
    ss1[:, 0::2] *= -1.0
    cc = np.tile(cc1, (B, 1))  # [64, 128]
    ss = np.tile(ss1, (B, 1))
    # columns ordered (bq, h, sq) to match the batch-major expn layout
    mask = np.zeros((B, S, B, QH, S), np.float32)
    for b in range(B):
        mask[b, :, b, :, :] = 1.0
    mask = mask.reshape(BS, QH * BS)
    if ROPE_BCAST:
        parts = [cc, ss, mask]
    else:
        parts = [np.tile(cc1, (B, QH)), np.tile(ss1, (B, QH)),
                 cc, ss, mask]
    return np.ascontiguousarray(
        np.concatenate(parts, axis=1)
    ).astype(bfloat16)


def _pmaj(w):
    # [KC*128, N] -> [128, KC*N]: per-partition-contiguous SBUF order
    kc, n = w.shape[0] // 128, w.shape[1]
    return np.ascontiguousarray(
        w.reshape(kc, 128, n).transpose(1, 0, 2).reshape(128, kc * n)
    ).astype(bfloat16)


def _wo_blocks(w):
    # [512, 4096] -> [128, 8(block), 4(chunk)*512]: column-block-major.
    # Block 6 is relaid half-major [2, 4, 256] and block 7 quarter-major
    # [4, 4, 128] so the tail's narrow loads stay DMA-contiguous.
    base = np.ascontiguousarray(
        w.reshape(4, 128, 8, 512).transpose(1, 2, 0, 3)
    )  # [128, 8, 4, 512]
    out = base.reshape(128, 8, 2048).copy()
    out[:, 6, :] = (
        base[:, 6].reshape(128, 4, 2, 256).transpose(0, 2, 1, 3)
        .reshape(128, 2048)
    )
    out[:, 7, :] = (
        base[:, 7].reshape(128, 4, 4, 128).transpose(0, 2, 1, 3)
        .reshape(128, 2048)
    )
    return out.astype(bfloat16)


def _v_pmaj(v):
    # [B, 2048, 128] -> [B, 128(p), NT*VW] bf16 with a ones column at HD
    # and zero pad to VW, so the DMA is one fully contiguous run
    vp = np.zeros((B, NT, 128, VW), np.float32)
    vp[:, :, :, :HD] = v.reshape(B, NT, 128, HD)
    vp[:, :, :, HD] = 1.0
    return np.ascontiguousarray(
        vp.transpose(0, 2, 1, 3).reshape(B, 128, NT * VW)
    ).astype(bfloat16)


_BASS_CACHE = {}


def make_in_maps(x, freqs_cos, freqs_sin, cache_k, cache_v, Wq, Wk, Wv, Wo):
    x = np.ascontiguousarray(np.asarray(x, np.float32))
    cache_k = np.asarray(cache_k, np.float32)
    cache_v = np.asarray(cache_v, np.float32)
    Wq = np.asarray(Wq, np.float32)
    Wk = np.asarray(Wk, np.float32)
    Wv = np.asarray(Wv, np.float32)
    Wo = np.asarray(Wo, np.float32)

    xT = np.ascontiguousarray(
        x.reshape(BS, KC, 128).transpose(2, 1, 0).reshape(128, KC * BS)
    ).astype(bfloat16)
    rope_cat = _rope_mask_tensor(freqs_cos, freqs_sin)
    idx = np.full((128, 8), -1, np.int16)
    for j in range(BS):
        idx[j % 16, j // 16] = j

    in_maps = []
    for c in range(NCORES):
        kc = cache_k[:, :START, c, :]  # [B, 2048, 128]
        in_maps.append(
            {
                "xT": xT,
                "wq": _pmaj(Wq[:, QW * c : QW * (c + 1)]),
                "wk": _pmaj(Wk[:, HD * c : HD * (c + 1)]),
                "wv": _pmaj(Wv[:, HD * c : HD * (c + 1)]),
                "wo": _wo_blocks(Wo[QW * c : QW * (c + 1), :]),
                "kT": np.ascontiguousarray(
                    kc.transpose(0, 2, 1)
                ).astype(bfloat16),
                "vc": _v_pmaj(cache_v[:, :START, c, :]),
                "rope": rope_cat,
                "idx16": idx,
            }
        )
    return in_maps


def kernel(x, freqs_cos, freqs_sin, cache_k, cache_v, Wq, Wk, Wv, Wo, start_pos):
    assert int(start_pos) == START
    in_maps = make_in_maps(x, freqs_cos, freqs_sin, cache_k, cache_v, Wq, Wk, Wv, Wo)
    if "nc" not in _BASS_CACHE:
        _BASS_CACHE["nc"] = build_bass()
    res = run_bass_kernel_spmd(
        _BASS_CACHE["nc"], in_maps, core_ids=list(range(NCORES))
    )
    total = np.zeros((BS, DIM), np.float32)
    for r in res.results:
        total += r["out"]
    return total.reshape(B, S, DIM)


# revision 59
# speedup vs baseline: 1.0021x; 1.0021x over previous
"""Trainium2 Bass kernel for single-step causal GQA attention with KV cache.

Problem (hardcoded shapes):
  x[4,16,4096] @ Wq/Wk/Wv -> RoPE -> append to KV cache (start_pos=2048) ->
  GQA attention over T=2064 keys -> @ Wo -> out[4,16,4096], all fp32.

Sharding (8 cores, tensor-parallel over heads):
  core c gets q-heads 4c..4c+3 (Wq cols c*512:(c+1)*512), kv-head c
  (Wk/Wv cols c*128:(c+1)*128, cache [:, :, c, :]), and Wo rows
  c*512:(c+1)*512 (row-parallel). The 8 partial [64,4096] outputs are
  summed on the host.

The kernel is DMA-bound (~15.5 MB/core of weights+cache), so everything
streamed from HBM is bf16: weights, caches, x, and rope tables are
converted on the host; matmuls run with bf16 operands (1 PE cycle/row vs
4 for fp32) accumulating into fp32 PSUM. All DMAs keep >=512B contiguous
runs (below that the bus pays 2x), which is why V is host-padded to
HD+4 columns with the softmax-denominator ones column baked in.

Per-core layout choices:
  - x is pre-transposed on host to xT[4096,64] so the QKV contraction
    (over DIM) streams W tiles as the moving operand with xT chunks as
    the 64-col stationary.
  - K cache is pre-transposed on host to kT[4,128,2048] so score matmuls
    use kT chunks [128(hd),128(t)] directly as stationary; scores come
    out transposed: scoresT[t, (h,s)] with keys on partitions.
  - softmax: exp without max-subtraction (|scores|<~10 for this data
    regime, exp is exact enough), and the denominator comes from a ones
    column in the V tiles, accumulated next to the unnormalized
    attention output, so no cross-partition reduction is ever needed.
  - RoPE is two multiplies and an add using host-precomputed cos/sin
    broadcast tiles, with the pair-swap expressed as a negative-stride AP.
"""

import math
import os

import numpy as np
from ml_dtypes import bfloat16

# SWDGE-scatter final flush (fast tail) vs plain HWDGE flushes (safe).
# The scatter path passes CoreSim/TimelineSim but hangs the real device
# (its DMA-completion sem wiring under walrus differs), so default off.
USE_SCATTER = os.environ.get("KERNEL_SCATTER", "0") == "1"
# zero-stride broadcast APs for the per-head rope tables vs host-materialized
ROPE_BCAST = os.environ.get("KERNEL_ROPE_BCAST", "1") == "1"
# strip the trailing sem-clear + second all-engine barrier from the exit
# block (all data work completes at the first barrier)
TRIM_TAIL = os.environ.get("KERNEL_TRIM_TAIL", "0") == "1"
# let SP skip the entry-block barrier (it only orders Pool's constant-tile
# memsets, which SP's DMA stream never touches) so the first HBM transfer
# starts ~0.55us earlier
FAST_HEAD = os.environ.get("KERNEL_FAST_HEAD", "0") == "1"

import concourse.bass as bass
import concourse.mybir as mybir
import concourse.tile as tile
from concourse import bacc
from concourse.bass_utils import run_bass_kernel_spmd
from concourse.masks import make_identity

F32 = mybir.dt.float32
BF16 = mybir.dt.bfloat16

B, S, DIM = 4, 16, 4096
NH, NKV, HD = 32, 8, 128
START = 2048
BS = B * S              # 64 tokens
NCORES = 8
QH = NH // NCORES       # 4 q heads per core
QW = QH * HD            # 512 = per-core Wq width
KC = DIM // 128         # 32 contraction chunks
NT = START // 128       # 16 full cache chunks per batch
TW = QH * S             # 64 = scoresT free width (h-major, then s)
EXPW = NT * TW          # 1024 = expT tile width (16 cache chunks)
VW = HD + 1             # padded V row: 128 v cols + ones col
SCALE = 1.0 / math.sqrt(HD)


def swap_pairs(ap):
    # [p, w] view with even/odd element pairs swapped: (x1,x0,x3,x2,...)
    p, w = ap.shape
    return bass.AP(ap.tensor, ap.offset + 1, [ap.ap[0], [2, w // 2], [-1, 2]])


def bcast_mid(ap2d, reps):
    # [p, w] view broadcast to [p, reps, w] via a zero-stride middle dim
    return bass.AP(ap2d.tensor, ap2d.offset, [ap2d.ap[0], [0, reps], ap2d.ap[1]])


def build_bass() -> bass.Bass:
    nc = bacc.Bacc()

    # xT is host-laid-out as [128(p), KC*64(c,s)] = exact SBUF order,
    # so the load is fully contiguous (8KB rows)
    xT = nc.dram_tensor("xT", [128, KC * BS], BF16, kind="ExternalInput")
    wq = nc.dram_tensor("wq", [128, KC * QW], BF16, kind="ExternalInput")
    wk = nc.dram_tensor("wk", [128, KC * HD], BF16, kind="ExternalInput")
    wv = nc.dram_tensor("wv", [128, KC * HD], BF16, kind="ExternalInput")
    wo = nc.dram_tensor("wo", [128, 8, 4 * 512], BF16, kind="ExternalInput")
    kT = nc.dram_tensor("kT", [B, HD, START], BF16, kind="ExternalInput")
    # V pre-padded on host to [.., NT*(HD+4)] with the ones column at HD
    vc = nc.dram_tensor("vc", [B, 128, NT * VW], BF16, kind="ExternalInput")
    # rope = [cc-base | ss-base | batch mask] merged into one load; with
    # ROPE_BCAST the per-head cc/ss broadcast happens via zero-stride APs
    # on chip, otherwise the host materializes the full-width tables
    ROPE_W = (2 * HD if ROPE_BCAST else 2 * QW + 2 * HD) + QH * BS
    rope = nc.dram_tensor("rope", [BS, ROPE_W], BF16, kind="ExternalInput")
    # scatter indices for the final out columns: row j at [j%16, j//16],
    # -1 elsewhere (scatter rows 64.. are skipped)
    idx16 = nc.dram_tensor("idx16", [128, 8], mybir.dt.int16,
                           kind="ExternalInput")
    out = nc.dram_tensor("out", [BS, DIM], BF16, kind="ExternalOutput")

    with tile.TileContext(nc) as tc:
        with (
            tc.tile_pool(name="const", bufs=1) as const,
            tc.tile_pool(name="wqp", bufs=8) as wqp,
            tc.tile_pool(name="wkvp", bufs=1) as wkvp,
            tc.tile_pool(name="wop", bufs=12) as wop,
            tc.tile_pool(name="kvp", bufs=4) as kvp,
            tc.tile_pool(name="acts", bufs=1) as acts,
            tc.tile_pool(name="expp", bufs=2) as expp,
            tc.tile_pool(name="small", bufs=4) as small,
        ):
            ident = const.tile([128, 64], F32, tag="ident")
            make_identity(nc, ident[:64, :])
            # touch Exp once so the ACT LUT loads during phase 1, not on
            # the batch-0 softmax critical path
            warm = const.tile([1, 4], F32, tag="warm")
            nc.scalar.activation(
                warm[:], ident[:1, :4], mybir.ActivationFunctionType.Exp
            )

            # DMA issue order matters: per-lane FIFO means earlier-needed
            # tiles must be enqueued first (xT, rope consts, wq stream);
            # wk/wv follow; kT/v and the big wo tiles go inside the batch
            # loop so they fill DMA idle time without delaying the start.
            xT_sb = const.tile([128, KC, BS], BF16, tag="xT")
            x4 = xT.ap()

            idx_sb = const.tile([128, 8], mybir.dt.int16, tag="idx16")
            rope_sb = const.tile([BS, ROPE_W], BF16, tag="rope")
            if ROPE_BCAST:
                cck_sb = rope_sb[:, 0:HD]
                ssk_sb = rope_sb[:, HD:2 * HD]
                mask_sb = rope_sb[:, 2 * HD:]      # [64, QH*64] batch mask
                ccq_sb = bcast_mid(cck_sb, QH)     # [64, QH, HD] broadcast
                ssq_sb = bcast_mid(ssk_sb, QH)
            else:
                ccq_sb = rope_sb[:, 0:QW]
                ssq_sb = rope_sb[:, QW:2 * QW]
                cck_sb = rope_sb[:, 2 * QW:2 * QW + HD]
                ssk_sb = rope_sb[:, 2 * QW + HD:2 * QW + 2 * HD]
                mask_sb = rope_sb[:, 2 * QW + 2 * HD:]

            wo_sb = []
            attnT = acts.tile([128, QH, BS], BF16, tag="attnT")

            with tc.tile_pool(name="ps_t", bufs=2, space="PSUM") as ps_t:
                # ---- QKV projections: psum[tok, width] += xT_c.T @ W_c ----
                with tc.tile_pool(name="ps_qkv", bufs=1, space="PSUM") as ps_qkv:
                    xq_ps = ps_qkv.tile([BS, QW], F32, tag="xq")
                    xk_ps = ps_qkv.tile([BS, HD], F32, tag="xk")
                    xv_ps = ps_qkv.tile([BS, HD], F32, tag="xv")

                    # interleave xT piece / wq group DMAs so the first
                    # matmul's inputs arrive early; 4 chunks per group keeps
                    # every contiguous run >= 512B in bf16
                    # wq group issued before its xT piece: the first (long)
                    # wq transfer covers the second DMA's descriptor-gen
                    # latency, so the bus never idles at the start
                    wq3 = wq.ap()
                    for g in range(8):  # 4 chunks per DMA group
                        wq_sb = wqp.tile([128, 4, QW], BF16, tag="wq")
                        nc.sync.dma_start(wq_sb[:], wq3[:, 2048 * g : 2048 * (g + 1)])
                        nc.sync.dma_start(
                            xT_sb[:, 4 * g : 4 * (g + 1), :],
                            x4[:, 256 * g : 256 * (g + 1)],
                        )
                        if g == 4:  # consts mid-stream: by then the bus is
                            # ahead of HWDGE descriptor-gen, so these two
                            # cheap DMAs don't delay the wq pipeline
                            nc.sync.dma_start(rope_sb[:], rope.ap())
                            nc.sync.dma_start(idx_sb[:], idx16.ap())
                        for j in range(4):
                            c = 4 * g + j
                            nc.tensor.matmul(
                                xq_ps[:],
                                lhsT=xT_sb[:, c, :],
                                rhs=wq_sb[:, j, :],
                                start=(c == 0),
                                stop=(c == KC - 1),
                            )

                    wk_sb = wkvp.tile([128, KC, HD], BF16, tag="wk")
                    nc.sync.dma_start(wk_sb[:], wk.ap())
                    wv_sb = wkvp.tile([128, KC, HD], BF16, tag="wv")
                    nc.sync.dma_start(wv_sb[:], wv.ap())
                    for c in range(KC):
                        nc.tensor.matmul(
                            xk_ps[:],
                            lhsT=xT_sb[:, c, :],
                            rhs=wk_sb[:, c, :],
                            start=(c == 0),
                            stop=(c == KC - 1),
                        )
                    for c in range(KC):
                        nc.tensor.matmul(
                            xv_ps[:],
                            lhsT=xT_sb[:, c, :],
                            rhs=wv_sb[:, c, :],
                            start=(c == 0),
                            stop=(c == KC - 1),
                        )

                    # ---- RoPE (token-major): o = x*cc + swap(x)*ss ----
                    xq_sb = acts.tile([BS, QW], F32, tag="xq_sb")
                    tq = acts.tile([BS, QW], F32, tag="tq")
                    nc.vector.tensor_mul(xq_sb[:], xq_ps[:], ccq_sb)
                    nc.vector.tensor_mul(tq[:], swap_pairs(xq_ps[:]), ssq_sb)
                    nc.vector.tensor_add(xq_sb[:], xq_sb[:], tq[:])

                    xk_sb = acts.tile([BS, HD], F32, tag="xk_sb")
                    tk = acts.tile([BS, HD], F32, tag="tk")
                    nc.vector.tensor_mul(xk_sb[:], xk_ps[:], cck_sb)
                    nc.vector.tensor_mul(tk[:], swap_pairs(xk_ps[:]), ssk_sb)
                    nc.vector.tensor_add(xk_sb[:], xk_sb[:], tk[:])

                    # v_new, token-major [64(b,s), HD | ones col]: used with a
                    # batch-masked exp so the contraction over all 64 token
                    # partitions only picks up the right batch's rows
                    xv1_sb = acts.tile([BS, HD + 1], BF16, tag="xv1")
                    nc.vector.tensor_copy(xv1_sb[:, :HD], xv_ps[:])
                    nc.vector.memset(xv1_sb[:, HD : HD + 1], 1.0)

                    # ---- transposes: qT [hd, (h | b,s)], kT_new [hd, (b,s)]
                    # (fp32 through PSUM; the copy out converts to bf16)
                    qT_sb = acts.tile([128, QH, TW], BF16, tag="qT")
                    for h in range(QH):
                        ps = ps_t.tile([128, BS], F32, tag="tr")
                        nc.tensor.transpose(
                            ps[:], xq_sb[:, 128 * h : 128 * (h + 1)],
                            ident[:BS, :BS],
                        )
                        nc.vector.tensor_copy(qT_sb[:, h, :], ps[:])
                    kTn_sb = acts.tile([128, BS], BF16, tag="kTn")
                    psn = ps_t.tile([128, BS], F32, tag="tr")
                    nc.tensor.transpose(psn[:], xk_sb[:], ident[:BS, :BS])
                    nc.vector.tensor_copy(kTn_sb[:], psn[:])

                    # ---- new-token scores for ALL batches in one matmul:
                    # scn_all[(bk,sk), (h,bq,sq)] then exp, then zero the
                    # bq != bk blocks with a host-precomputed 0/1 mask.
                    # expn is laid out batch-major so each batch's lhsT
                    # slice is one contiguous free dim (walrus requires
                    # single-free-dim stationary APs); exp runs per batch
                    # reading the (h, b, s)-ordered scores via a strided AP.
                    scn_ps = ps_qkv.tile([BS, QH * BS], F32, tag="scn")
                    nc.tensor.matmul(
                        scn_ps[:],
                        lhsT=kTn_sb[:],
                        rhs=qT_sb[:, :, :],
                        start=True,
                        stop=True,
                    )
                    expn = acts.tile([BS, B, QH * S], BF16, tag="expn")
                    scn_ap = scn_ps[:]
                    for b in range(B):
                        scn_b = bass.AP(
                            scn_ap.tensor, scn_ap.offset + S * b,
                            [scn_ap.ap[0], [TW, QH], [1, S]],
                        )
                        nc.scalar.activation(
                            expn[:, b, :], scn_b,
                            mybir.ActivationFunctionType.Exp,
                            scale=SCALE,
                        )
                    nc.vector.tensor_mul(expn[:], expn[:], mask_sb)

                # ---- attention per batch ----
                with (
                    tc.tile_pool(name="ps_sc", bufs=3, space="PSUM") as ps_sc,
                    tc.tile_pool(name="ps_ou", bufs=2, space="PSUM") as ps_ou,
                ):
                    def load_kv(b):
                        kT_sb = kvp.tile([128, START], BF16, tag="kT",
                                         name="kT_sb")
                        nc.sync.dma_start(kT_sb[:], kT.ap()[b])
                        v_sb = kvp.tile([128, NT * VW], BF16, tag="v",
                                        name="v_sb")
                        nc.sync.dma_start(v_sb[:], vc.ap()[b])
                        return kT_sb, v_sb

                    # all four batches' K/V prefetch up front (kvp bufs=4):
                    # the late batches' chains are the kernel tail, so their
                    # data must never wait behind the Wo prefetches
                    kv_tiles = {b: load_kv(b) for b in range(B)}
                    for b in range(B):
                        kT_sb, v_sb = kv_tiles.pop(b)

                        qT_b = qT_sb[:, :, 16 * b : 16 * (b + 1)]  # [128,4,16]

                        expT = expp.tile([128, EXPW], BF16, tag="expT")
                        for u in range(NT // 4):  # one exp per 4 chunks
                            sc = ps_sc.tile([128, 4, TW], F32, tag="sc")
                            for j in range(4):
                                t = 4 * u + j
                                nc.tensor.matmul(
                                    sc[:, j, :],
                                    lhsT=kT_sb[:, 128 * t : 128 * (t + 1)],
                                    rhs=qT_b,
                                    start=True,
                                    stop=True,
                                )
                            nc.scalar.activation(
                                expT[:, 4 * TW * u : 4 * TW * (u + 1)],
                                sc[:],
                                mybir.ActivationFunctionType.Exp,
                                scale=SCALE,
                            )
                        # unnormalized out [tok(h,s), hd | exp-sum col at HD]
                        ou = ps_ou.tile([TW, VW], F32, tag="ou")
                        for t in range(NT):
                            nc.tensor.matmul(
                                ou[:, : HD + 1],
                                lhsT=expT[:, TW * t : TW * (t + 1)],
                                rhs=v_sb[:, VW * t : VW * t + HD + 1],
                                start=(t == 0),
                                stop=False,
                            )
                        nc.tensor.matmul(
                            ou[:, : HD + 1],
                            lhsT=expn[:, b, :],
                            rhs=xv1_sb[:],
                            start=False,
                            stop=True,
                        )

                        rcp = small.tile([TW, 1], F32, tag="rcp")
                        nc.vector.reciprocal(rcp[:], ou[:, HD : HD + 1])
                        attn = small.tile([TW, HD], F32, tag="attn")
                        nc.vector.tensor_scalar_mul(attn[:], ou[:, :HD], rcp[:])

                        aps = ps_t.tile([128, TW], F32, tag="tr")
                        nc.tensor.transpose(aps[:], attn[:], ident[:TW, :TW])
                        # one strided copy: dst (h, s) columns <- src h-major
                        nc.vector.tensor_copy(
                            attnT[:, :, 16 * b : 16 * (b + 1)], aps[:]
                        )

                        # Wo prefetch as column blocks, two per batch; the
                        # final blocks narrow progressively (block 6 halves,
                        # block 7 quarters, host-relaid to stay contiguous)
                        # so the last-arriving bytes have the shortest
                        # possible downstream chain (4 matmuls of N=128)
                        if b < 3:
                            for j in (2 * b, 2 * b + 1):
                                wo_t = wop.tile([128, 4, 512], BF16, tag="wo",
                                                name="wo_t")
                                nc.sync.dma_start(wo_t[:], wo.ap()[:, j, :])
                                wo_sb.append(wo_t)
                        else:
                            for h in (0, 1):
                                wo_t = wop.tile([128, 4, 256], BF16,
                                                tag="wo", name="wo_t")
                                nc.sync.dma_start(
                                    wo_t[:],
                                    wo.ap()[:, 6, 1024 * h : 1024 * (h + 1)],
                                )
                                wo_sb.append(wo_t)
                            for h in range(4):
                                wo_t = wop.tile([128, 4, 128], BF16,
                                                tag="wo", name="wo_t")
                                nc.sync.dma_start(
                                    wo_t[:],
                                    wo.ap()[:, 7, 512 * h : 512 * (h + 1)],
                                )
                                wo_sb.append(wo_t)

            # ---- output projection: out[64, 4096] = attnT.T @ Wo ----
            # n-outer: out n-tile j consumes only column block j, freeing
            # its slot for blocks 6/7 to load as soon as attnT completes
            with (
                tc.tile_pool(name="outp", bufs=2) as outp,
                tc.tile_pool(name="ps_wo", bufs=3, space="PSUM") as ps_wo,
            ):
                # (block, n-columns within out, width)
                pieces = [(i, 512 * i, 512) for i in range(6)] + [
                    (6, 3072, 256), (7, 3328, 256),
                    (8, 3584, 128), (9, 3712, 128),
                    (10, 3840, 128), (11, 3968, 128),
                ]
                o_halves = [
                    outp.tile([BS, 2048], BF16, tag="o", name="o_sb")
                    for _ in range(2)
                ]
                # Columns [3072:4096] go out via a pre-prepared SWDGE scatter
                # (64 row descriptors of 2KB, fired by a cheap Pool trigger
                # after the last copy) instead of an HWDGE dma_start — this
                # skips the ~1.3us descriptor-gen pipeline that would
                # otherwise sit on the critical path after the last Wo bytes.
                if USE_SCATTER:
                    stage = outp.tile([128, 1, 1024], BF16, tag="stage")
                    nc.gpsimd.memset(stage[:], 0.0)
                    scat_sem = nc.alloc_semaphore("scat_done")
                    nc.gpsimd.dma_scatter_add(
                        out.ap()[:, 3072:4096],
                        stage[:],
                        idx_sb[:, :4],
                        num_idxs=BS,
                        num_idxs_reg=BS,
                        elem_size=1024,
                        elem_step=DIM,
                        prepare_only=True,
                        sem=scat_sem,
                    )
                for i, (blk, col, w) in enumerate(pieces):
                    wo_ps = ps_wo.tile([BS, 512], F32, tag="wops", name="wo_ps")
                    for k in range(4):
                        nc.tensor.matmul(
                            wo_ps[:, :w],
                            lhsT=attnT[:, k, :],
                            rhs=wo_sb[blk][:, k, :],
                            start=(k == 0),
                            stop=(k == 3),
                        )
                    eng = nc.vector.tensor_copy if i % 2 == 0 else nc.scalar.copy
                    if USE_SCATTER and col >= 3072:
                        eng(stage[:BS, 0, col - 3072 : col - 3072 + w],
                            wo_ps[:, :w])
                    else:
                        o_sb = o_halves[col // 2048]
                        eng(o_sb[:, col % 2048 : col % 2048 + w], wo_ps[:, :w])
                    if col + w == 2048:
                        nc.sync.dma_start(out.ap()[:, :2048], o_sb[:])
                    elif col + w == 3072:
                        nc.sync.dma_start(
                            out.ap()[:, 2048:3072], o_sb[:, :1024]
                        )
                    elif col + w == 3584 and not USE_SCATTER:
                        nc.sync.dma_start(
                            out.ap()[:, 3072:3584], o_sb[:, 1024:1536]
                        )
                    elif col + w == 3840 and not USE_SCATTER:
                        nc.sync.dma_start(
                            out.ap()[:, 3584:3840], o_sb[:, 1536:1792]
                        )
                    elif col + w == 4096:
                        if USE_SCATTER:
                            nc.gpsimd.trigger_dma(count=None)
                        else:
                            nc.sync.dma_start(
                                out.ap()[:, 3840:4096], o_sb[:, 1792:]
                            )
                if USE_SCATTER:
                    nc.gpsimd.wait_ge(scat_sem, 16)

    nc.compile()

    # Tile's teardown drain waits on the SWDGE lane sem (DMASW0) that the
    # prepared scatter's tick advanced — but a gen_mode=1 prep's completion
    # fires its custom sem= instead, so that wait can never be satisfied
    # (framework gap). The explicit gpsimd.wait_ge(scat_sem) above already
    # holds the final barrier until the scatter lands, so the orphaned
    # DMASW wait is redundant: drop it.
    if USE_SCATTER:
        patched = 0
        for blk in nc.m.functions[0].blocks:
            for inst in blk.instructions:
                si = inst.sync_info
                if si is None:
                    continue
                waits = list(si.on_wait)
                kept = [
                    w for w in waits
                    if not (w.ant_name or "").startswith("DMASW")
                ]
                if len(kept) != len(waits):
                    si.on_wait = kept
                    patched += 1
        assert patched == 1, (
            f"expected exactly one DMASW drain wait, {patched=}"
        )

    if FAST_HEAD:
        entry_blk = nc.m.functions[0].blocks[0]
        neutered = 0
        for inst in entry_blk.instructions:
            if (inst.engine == mybir.EngineType.SP
                    and inst.name.startswith("barrier_SP")):
                si = inst.sync_info
                if si is not None and si.on_wait:
                    si.on_wait = []
                    neutered += 1
        assert neutered == 1, f"expected one SP entry barrier, {neutered=}"

    if TRIM_TAIL:
        # The exit block ends with: [first barrier] -> Pool sem-clear (ISA)
        # -> [second all-engine barrier]. All data guarantees are complete
        # at the first barrier (SP's drain holds the DMA-completion waits),
        # and the sem-clear leaves every semaphore at zero — so the second
        # barrier's waits/updates are pure tail latency. Neuter them into
        # sequencer no-ops (keeping sems at zero for re-execution hygiene).
        exit_blk = nc.m.functions[0].blocks[-1]
        insts = list(exit_blk.instructions)
        isa_idx = max(
            i for i, inst in enumerate(insts)
            if type(inst).__name__ == "InstISA"
        )
        for inst in insts[isa_idx + 1:]:
            si = inst.sync_info
            if si is not None:
                si.on_wait = []
                si.on_update = []
    return nc


def _rope_mask_tensor(freqs_cos, freqs_sin):
    # cc/ss rope tiles (row r=(b*16+s), col 2i+j; o = x*cc + swap(x)*ss
    # with cc=[c,c,...], ss=[-s,+s,...]) then the batch mask
    # m[(bk,sk),(h,bq,sq)] = (bk==bq).
    cos = np.asarray(freqs_cos, np.float32)  # [S, 64]
    sin = np.asarray(freqs_sin, np.float32)
    cc1 = np.repeat(cos, 2, axis=1)  # [S, 128]
    ss1 = np.repeat(sin, 2, axis=1).copy()
    ss1[:, 0::2] *= -1.0
    cc = np.tile(cc1, (B, 1))  # [64, 128]
    ss = np.tile(ss1, (B, 1))
    # columns ordered (bq, h, sq) to match the batch-major expn layout
    mask = np.zeros((B, S, B, QH, S), np.float32)
    for b in range(B):
        mask[b, :, b, :, :] = 1.0
    mask = mask.reshape(BS, QH * BS)
    if ROPE_BCAST:
        parts = [cc, ss, mask]
    else:
        parts = [np.tile(cc1, (B, QH)), np.tile(ss1, (B, QH)),
                 cc, ss, mask]
    return np.ascontiguousarray(
        np.concatenate(parts, axis=1)
    ).astype(bfloat16)


def _pmaj(w):
    # [KC*128, N] -> [128, KC*N]: per-partition-contiguous SBUF order
    kc, n = w.shape[0] // 128, w.shape[1]
    return np.ascontiguousarray(
        w.reshape(kc, 128, n).transpose(1, 0, 2).reshape(128, kc * n)
    ).astype(bfloat16)


def _wo_blocks(w):
    # [512, 4096] -> [128, 8(block), 4(chunk)*512]: column-block-major.
    # Block 6 is relaid half-major [2, 4, 256] and block 7 quarter-major
    # [4, 4, 128] so the tail's narrow loads stay DMA-contiguous.
    base = np.ascontiguousarray(
        w.reshape(4, 128, 8, 512).transpose(1, 2, 0, 3)
    )  # [128, 8, 4, 512]
    out = base.reshape(128, 8, 2048).copy()
    out[:, 6, :] = (
        base[:, 6].reshape(128, 4, 2, 256).transpose(0, 2, 1, 3)
        .reshape(128, 2048)
    )
    out[:, 7, :] = (
        base[:, 7].reshape(128, 4, 4, 128).transpose(0, 2, 1, 3)
        .reshape(128, 2048)
    )
    return out.astype(bfloat16)


def _v_pmaj(v):
    # [B, 2048, 128] -> [B, 128(p), NT*VW] bf16 with a ones column at HD
    # and zero pad to VW, so the DMA is one fully contiguous run
    vp = np.zeros((B, NT, 128, VW), np.float32)
    vp[:, :, :, :HD] = v.reshape(B, NT, 128, HD)
    vp[:, :, :, HD] = 1.0
    return np.ascontiguousarray(
        vp.transpose(0, 2, 1, 3).reshape(B, 128, NT * VW)
    ).astype(bfloat16)


_BASS_CACHE = {}


def make_in_maps(x, freqs_cos, freqs_sin, cache_k, cache_v, Wq, Wk, Wv, Wo):
    x = np.ascontiguousarray(np.asarray(x, np.float32))
    cache_k = np.asarray(cache_k, np.float32)
    cache_v = np.asarray(cache_v, np.float32)
    Wq = np.asarray(Wq, np.float32)
    Wk = np.asarray(Wk, np.float32)
    Wv = np.asarray(Wv, np.float32)
    Wo = np.asarray(Wo, np.float32)

    xT = np.ascontiguousarray(
        x.reshape(BS, KC, 128).transpose(2, 1, 0).reshape(128, KC * BS)
    ).astype(bfloat16)
    rope_cat = _rope_mask_tensor(freqs_cos, freqs_sin)
    idx = np.full((128, 8), -1, np.int16)
    for j in range(BS):
        idx[j % 16, j // 16] = j

    in_maps = []
    for c in range(NCORES):
        kc = cache_k[:, :START, c, :]  # [B, 2048, 128]
        in_maps.append(
            {
                "xT": xT,
                "wq": _pmaj(Wq[:, QW * c : QW * (c + 1)]),
                "wk": _pmaj(Wk[:, HD * c : HD * (c + 1)]),
                "wv": _pmaj(Wv[:, HD * c : HD * (c + 1)]),
                "wo": _wo_blocks(Wo[QW * c : QW * (c + 1), :]),
                "kT": np.ascontiguousarray(
                    kc.transpose(0, 2, 1)
                ).astype(bfloat16),
                "vc": _v_pmaj(cache_v[:, :START, c, :]),
                "rope": rope_cat,
                "idx16": idx,
            }
        )
    return in_maps


def kernel(x, freqs_cos, freqs_sin, cache_k, cache_v, Wq, Wk, Wv, Wo, start_pos):
    assert int(start_pos) == START
    in_maps = make_in_maps(x, freqs_cos, freqs_sin, cache_k, cache_v, Wq, Wk, Wv, Wo)
    if "nc" not in _BASS_CACHE:
        _BASS_CACHE["nc"] = build_bass()
    res = run_bass_kernel_spmd(
        _BASS_CACHE["nc"], in_maps, core_ids=list(range(NCORES))
    )
    total = np.zeros((BS, DIM), np.float32)
    for r in res.results:
        total += r["out"]
    return total.reshape(B, S, DIM)
